# revision 21
# baseline (speedup 1.0000x reference)
"""MLA-style attention (nn_CausalSelfAttention_34626026341018) for 8 trn2 cores.

Shapes (hardcoded): B=4, T=2048, C=1024, H=16, HD=64, KV=64, QL=512.
Sharding: core c handles batch b=c//2, query half = c%2 (1024 queries), full
key range. Inputs are pre-transposed/rotated on host; every core runs the
identical program on its own data.

Key structural ideas vs the straightforward formulation:
  - wv_u is folded into w_proj on the host (MLA absorption): the kernel
    computes Z_h = softmax(S_h) @ c_kv (64-dim latent) and one final GEMM
    with mproj[h*64+k, o] = sum_d wv_u[h*64+d, k] * w_proj[o, h*64+d].
    bv_u folds into b_proj. The V up-projection disappears.
  - softmax rowsums come free from a ones column appended to the c_kv
    lhsT (row 64 of the Z accumulator), removing the ones-matmuls.
  - exp is split across the scalar engine (true Exp) and the vector
    engine (Schraudolph fp16 bit-trick: int16 <- S*A + B, bitcast f16),
    which otherwise bottlenecks the whole kernel on ACT.

Per-core pipeline:
  c_q^T [QL,Tq], ckv^T [64,T] (fp16), q^T [H*KV,Tq] (fp16)
  ckv65 [128, 16*65] = per-s-block transposed c_kv with ones column
  per (head-pair j, query-chunk n of 512, s-block i of 128):
      S pair [128,1024] psum <- 2 matmuls (shared lhsT)
      P = exp(S*0.125) fp16 via ACT or DVE (alternating)
      Z'A [65,512], Z'B [65,512] psum accumulate over i (row 64 = rowsum)
  tail: r = recip(rowsum), broadcast via K=1 matmul, y = Z' * r
  out = Y^T.T @ mproj + b~proj
"""

import numpy as np

import concourse.bass as bass
import concourse.mybir as mybir
import concourse.tile as tile
from concourse import bacc
from concourse.bass_utils import run_bass_kernel_spmd

F32 = mybir.dt.float32
F32R = mybir.dt.float32r
F16 = mybir.dt.float16
BF16 = mybir.dt.bfloat16
I16 = mybir.dt.int16
AF = mybir.ActivationFunctionType
ALU = mybir.AluOpType

B, T, C = 4, 2048, 1024
H, HD, KV, QL = 16, 64, 64, 512
P = 128
TQ = 1024              # queries per core
KC = C // P            # 8 k-chunks over C
MQL = QL // P          # 4 m-tiles of c_q^T
KQL = QL // P          # 4 k-chunks over QL
MH = (H * KV) // P     # 8 m-tiles of q^T (= head pairs)
PAIRS = H // 2         # 8
TKT = T // P           # 16 s-blocks
NQ = TQ // 512         # 2 Tq chunks of 512

# Schraudolph fp16 exp: bits = round(S * SCH_A + SCH_B); bitcast as fp16
# approximates exp(S * 0.125) with max rel err ~3%, zero-mean after softmax.
SCH_A = 184.6649652337873
SCH_B = 15315.5

# i-blocks whose exp runs on ACT (9); the rest run on DVE (7)
ACT_I = frozenset([0, 2, 4, 6, 8, 10, 12, 14, 15])

_CACHE = {}


class _nullctx:
    def __enter__(self):
        return None

    def __exit__(self, *a):
        return False


def build_nc(reps=1):
    nc = bacc.Bacc(None, target_bir_lowering=False)

    xT = nc.dram_tensor("xT", [C, T], F32R, kind="ExternalInput")
    wq_dT = nc.dram_tensor("wq_dT", [C, QL], F32R, kind="ExternalInput")
    w_qkT = nc.dram_tensor("w_qkT", [QL, H * KV], F32R, kind="ExternalInput")
    wkv_dT = nc.dram_tensor("wkv_dT", [C, KV], F32R, kind="ExternalInput")
    mproj = nc.dram_tensor("mproj", [H * KV, C], F32R, kind="ExternalInput")
    bq_p = nc.dram_tensor("bq_p", [P, MQL], F32, kind="ExternalInput")
    bqk_p = nc.dram_tensor("bqk_p", [P, MH], F32, kind="ExternalInput")
    bkv_d = nc.dram_tensor("bkv_d", [KV, 1], F32, kind="ExternalInput")
    bproj_bc = nc.dram_tensor("bproj_bc", [P, C], F32, kind="ExternalInput")
    ones_d = nc.dram_tensor("ones_d", [1, 64], F32R, kind="ExternalInput")
    ident_d = nc.dram_tensor("ident_d", [64, 64], F16, kind="ExternalInput")
    out = nc.dram_tensor("out", [TQ, C], F32, kind="ExternalOutput")

    with tile.TileContext(nc) as tc:
      with (tc.For_i(0, reps, 1) if reps > 1 else _nullctx()):
        with tc.tile_pool(name="persist", bufs=1) as pp:
            ones_sb = pp.tile([1, 64], F32R, name="ones_sb")
            ident_sb = pp.tile([64, 64], F16, name="ident_sb")
            bq_sb = pp.tile([P, MQL], F32, name="bq_sb")
            bqk_sb = pp.tile([P, MH], F32, name="bqk_sb")
            bkv_sb = pp.tile([KV, 1], F32, name="bkv_sb")
            bproj_sb = pp.tile([P, C], F32, name="bproj_sb")
            nc.sync.dma_start(ones_sb[:], ones_d[:])
            nc.sync.dma_start(ident_sb[:], ident_d[:])
            nc.sync.dma_start(bq_sb[:], bq_p[:])
            nc.sync.dma_start(bqk_sb[:], bqk_p[:])
            nc.sync.dma_start(bkv_sb[:], bkv_d[:])
            nc.sync.dma_start(bproj_sb[:], bproj_bc[:])

            ckvT = pp.tile([P, T], F16, name="ckvT")  # rows 64:128 duplicate 0:64
            ckv65 = pp.tile([P, TKT * 65], F16, name="ckv65")
            q_sb = [pp.tile([P, TQ], F16, name=f"q{m}") for m in range(MH)]
            y_sb = [pp.tile([P, TQ], F32R, name=f"y{j}") for j in range(PAIRS)]

            # ---------------- stage A ----------------
            with (
                tc.tile_pool(name="stA", bufs=1) as sa,
                tc.tile_pool(name="psA", bufs=3, space="PSUM") as psA,
                tc.tile_pool(name="psT", bufs=2, space="PSUM") as psT,
            ):
                xT_sb = [sa.tile([P, T], F32R, name=f"x{k}") for k in range(KC)]
                wq_sb = [sa.tile([P, QL], F32R, name=f"wq{k}") for k in range(KC)]
                wkv_sb = [sa.tile([P, KV], F32R, name=f"wkv{k}") for k in range(KC)]
                for k in range(KC):
                    nc.sync.dma_start(wkv_sb[k][:], wkv_dT[k * P:(k + 1) * P, :])
                for k in range(KC):
                    nc.sync.dma_start(xT_sb[k][:], xT[k * P:(k + 1) * P, :])
                for k in range(KC):
                    nc.sync.dma_start(wq_sb[k][:], wq_dT[k * P:(k + 1) * P, :])

                # ckv^T [64, T] fp16
                for ns in range(T // 512):
                    sl = slice(ns * 512, (ns + 1) * 512)
                    ckv_ps = psA.tile([KV, 512], F32, tag="ps", name="ckv_ps")
                    for k in range(KC):
                        nc.tensor.matmul(
                            ckv_ps[:], wkv_sb[k][:], xT_sb[k][:, sl],
                            start=(k == 0), stop=(k == KC - 1))
                    nc.scalar.activation(ckvT[0:64, sl], ckv_ps[:], AF.Identity,
                                         bias=bkv_sb[:, 0:1])
                nc.sync.dma_start(ckvT[64:128, :], ckvT[0:64, :])

                # ckv65: transposed s-blocks with a trailing ones column
                # block i: col 0 = ones (-> rowsum in Z row 0), cols 1:65 = ckv
                nc.vector.memset(ckv65[:], 1.0)
                for i in range(TKT):
                    tp = psT.tile([P, 64], F16, tag="tp", name="tp")
                    nc.tensor.transpose(
                        tp[:], ckvT[0:64, i * P:(i + 1) * P], ident_sb[:])
                    nc.vector.tensor_copy(ckv65[:, i * 65:i * 65 + 64], tp[:])

                # c_q^T [QL, Tq] f32r
                cq_sb = [sa.tile([P, TQ], F32R, name=f"cq{m}") for m in range(MQL)]
                for m in range(MQL):
                    msl = slice(m * P, (m + 1) * P)
                    for n in range(NQ):
                        nsl = slice(n * 512, (n + 1) * 512)
                        cq_ps = psA.tile([P, 512], F32, tag="ps", name="cq_ps")
                        for k in range(KC):
                            nc.tensor.matmul(
                                cq_ps[:], wq_sb[k][:, msl], xT_sb[k][:, nsl],
                                start=(k == 0), stop=(k == KC - 1))
                        nc.scalar.activation(cq_sb[m][:, nsl], cq_ps[:],
                                             AF.Identity, bias=bq_sb[:, m:m + 1])

                # q^T [H*KV, Tq] fp16
                wqk_sb = [sa.tile([P, H * KV], F32R, name=f"wqk{k}")
                          for k in range(KQL)]
                for k in range(KQL):
                    nc.sync.dma_start(wqk_sb[k][:], w_qkT[k * P:(k + 1) * P, :])
                for m in range(MH):
                    msl = slice(m * P, (m + 1) * P)
                    for n in range(NQ):
                        nsl = slice(n * 512, (n + 1) * 512)
                        q_ps = psA.tile([P, 512], F32, tag="ps", name="q_ps")
                        for k in range(KQL):
                            nc.tensor.matmul(
                                q_ps[:], wqk_sb[k][:, msl], cq_sb[k][:, nsl],
                                start=(k == 0), stop=(k == KQL - 1))
                        nc.scalar.activation(q_sb[m][:, nsl], q_ps[:],
                                             AF.Identity, bias=bqk_sb[:, m:m + 1])

            # ---------------- stage B ----------------
            with tc.tile_pool(name="mp", bufs=1) as mpool:
                mproj_sb = [mpool.tile([P, C], F32R, name=f"mp{j}")
                            for j in range(MH)]
                for j in range(MH):
                    nc.sync.dma_start(mproj_sb[j][:], mproj[j * P:(j + 1) * P, :])

                with (
                    tc.tile_pool(name="pP", bufs=3) as pP,
                    tc.tile_pool(name="pR", bufs=2) as pR,
                    tc.tile_pool(name="psS", bufs=2, space="PSUM") as psS,
                    tc.tile_pool(name="psZ", bufs=2, space="PSUM") as psZ,
                ):
                    # Tail (normalize) ops for unit u are emitted interleaved
                    # into unit u+1's i-loop so they never stall the strict
                    # FIFO queues of ACT/DVE: by the time each queue reaches
                    # them, their dependencies are long satisfied.
                    def tail_stage(st, prev):
                        jp, nslp, zAp, zBp, tl = prev
                        if st == 0:
                            tl["rs"] = pR.tile([1, 1024], F32, tag="rs",
                                               name="rs_sb")
                            nc.scalar.activation(tl["rs"][:, 0:512],
                                                 zAp[64:65, :], AF.Copy)
                            nc.scalar.activation(tl["rs"][:, 512:1024],
                                                 zBp[64:65, :], AF.Copy)
                        elif st == 1:
                            tl["r"] = pR.tile([1, 1024], F32, tag="r",
                                              name="r_sb")
                            nc.vector.reciprocal_approx_fast(
                                tl["r"][:, 0:512], tl["rs"][:, 0:512])
                            nc.vector.reciprocal_approx_fast(
                                tl["r"][:, 512:1024], tl["rs"][:, 512:1024])
                        elif st == 2:
                            tl["rbc"] = pR.tile([64, 1024], F32, tag="rbc",
                                                name="r_bc")
                            nc.gpsimd.partition_broadcast(tl["rbc"][:],
                                                          tl["r"][:])
                        elif st == 3:
                            nc.vector.tensor_tensor(
                                y_sb[jp][0:64, nslp], zAp[0:64, :],
                                tl["rbc"][:, 0:512], ALU.mult)
                        elif st == 4:
                            nc.vector.tensor_tensor(
                                y_sb[jp][64:128, nslp], zBp[0:64, :],
                                tl["rbc"][:, 512:1024], ALU.mult)

                    TRIG = {2: 0, 4: 1, 6: 2, 10: 3, 11: 4}
                    prev = None
                    for j in range(PAIRS):
                        for n in range(NQ):
                            nsl = slice(n * 512, (n + 1) * 512)
                            zA = psZ.tile([65, 512], F32, tag="zA", name="zA")
                            zB = psZ.tile([65, 512], F32, tag="zB", name="zB")

                            def emit_S(i):
                                ssl = slice(i * P, (i + 1) * P)
                                SB_ = psS.tile([P, 512], F32, tag="SB",
                                               name="SB_")
                                SA_ = psS.tile([P, 512], F32, tag="SA",
                                               name="SA_")
                                nc.tensor.matmul(
                                    SB_[:], ckvT[64:128, ssl],
                                    q_sb[j][64:128, nsl], start=True, stop=True)
                                nc.tensor.matmul(
                                    SA_[:], ckvT[0:64, ssl],
                                    q_sb[j][0:64, nsl], start=True, stop=True)
                                return SA_, SB_

                            def emit_Z(i, PtA, PtB):
                                csl = slice(i * 65, (i + 1) * 65)
                                nc.tensor.matmul(
                                    zB[:], ckv65[:, csl], PtB[:],
                                    start=(i == 0), stop=(i == TKT - 1))
                                nc.tensor.matmul(
                                    zA[:], ckv65[:, csl], PtA[:],
                                    start=(i == 0), stop=(i == TKT - 1))

                            s_next = emit_S(0)
                            z_pend = None
                            for i in range(TKT):
                                SA_, SB_ = s_next
                                PtB = pP.tile([P, 512], F16, tag="PtB",
                                              name="PtB")
                                nc.vector.tensor_scalar(
                                    PtB[:].bitcast(I16), SB_[:],
                                    SCH_A, SCH_B, ALU.mult, ALU.add)
                                PtA = pP.tile([P, 512], F16, tag="PtA",
                                              name="PtA")
                                nc.scalar.activation(PtA[:], SA_[:], AF.Exp,
                                                     scale=0.125)
                                if i + 1 < TKT:
                                    s_next = emit_S(i + 1)
                                if z_pend is not None:
                                    emit_Z(*z_pend)
                                z_pend = (i, PtA, PtB)
                                if prev is not None and i in TRIG:
                                    tail_stage(TRIG[i], prev)
                            emit_Z(*z_pend)
                            prev = (j, nsl, zA, zB, {})
                    for st in range(5):
                        tail_stage(st, prev)

                # ---------------- proj ----------------
                with (
                    tc.tile_pool(name="pO", bufs=2) as pO,
                    tc.tile_pool(name="psO", bufs=4, space="PSUM") as psO,
                ):
                    for tt in range(TQ // P):
                        tsl = slice(tt * P, (tt + 1) * P)
                        o_sb = pO.tile([P, C], F32, name="o_sb")
                        for oc in range(C // 512):
                            osl = slice(oc * 512, (oc + 1) * 512)
                            o_ps = psO.tile([P, 512], F32, name="o_ps")
                            for k in range(MH):
                                nc.tensor.matmul(
                                    o_ps[:], y_sb[k][:, tsl],
                                    mproj_sb[k][:, osl],
                                    start=(k == 0), stop=(k == MH - 1))
                            nc.vector.tensor_tensor(
                                o_sb[:, osl], o_ps[:], bproj_sb[:, osl],
                                ALU.add)
                        nc.sync.dma_start(out[tsl, :], o_sb[:])

    nc.compile()
    return nc


def _prep_maps(x, wq_d, bq_d, w_qk, b_qk, wkv_d, bkv_d, wv_u, bv_u, w_proj, b_proj):
    f = np.float32
    wv3 = np.asarray(wv_u, f).reshape(H, HD, KV)
    wp3 = np.asarray(w_proj, f).reshape(C, H, HD)
    mproj = np.einsum('hdk,ohd->hko', wv3, wp3).reshape(H * KV, C)
    bproj = np.asarray(b_proj, f) + np.asarray(w_proj, f) @ np.asarray(bv_u, f)
    shared = {
        "wq_dT": np.ascontiguousarray(np.asarray(wq_d, f).T),
        "w_qkT": np.ascontiguousarray(np.asarray(w_qk, f).T),
        "wkv_dT": np.ascontiguousarray(np.asarray(wkv_d, f).T),
        "mproj": np.ascontiguousarray(mproj),
        "bq_p": np.ascontiguousarray(np.asarray(bq_d, f).reshape(MQL, P).T),
        "bqk_p": np.ascontiguousarray(np.asarray(b_qk, f).reshape(MH, P).T),
        "bkv_d": np.asarray(bkv_d, f).reshape(KV, 1),
        "bproj_bc": np.broadcast_to(bproj, (P, C)).copy(),
        "ones_d": np.ones((1, 64), f),
        "ident_d": np.eye(64, dtype=np.float16),
    }
    in_maps = []
    for c in range(8):
        b, half = divmod(c, 2)
        xTb = np.ascontiguousarray(np.asarray(x[b], f).T)  # [C, T]
        if half:
            xTb = np.ascontiguousarray(
                np.concatenate([xTb[:, TQ:], xTb[:, :TQ]], axis=1))
        m = dict(shared)
        m["xT"] = xTb
        in_maps.append(m)
    return in_maps


def kernel(**inputs):
    if "nc" not in _CACHE:
        _CACHE["nc"] = build_nc()
    nc = _CACHE["nc"]
    in_maps = _prep_maps(**inputs)
    res = run_bass_kernel_spmd(nc, in_maps, core_ids=list(range(8)))
    _CACHE["last_result"] = res
    y = np.empty((B, T, C), dtype=np.float32)
    for c in range(8):
        b, half = divmod(c, 2)
        y[b, half * TQ:(half + 1) * TQ, :] = res.results[c]["out"]
    return y
